# revision 4
# baseline (speedup 1.0000x reference)
"""EvolveGCN (2-layer GCN + GRU weight evolution) on 8 Trainium2 NeuronCores.

Sharding: nodes/edges by destination core; GRU tensor-parallel over the
gate dim; GCN weights replicated.

  - Host pre-scales node embeddings by out_norm (f32); layer-1 gathers read
    them directly (no device x-scaling pass).
  - SpMM per core: f32 dma_gather (4 parallel SWDGE queues, indices
    sorted by target address for HBM locality) into 128-slot chunks,
    then "staircase" segment matmuls into PSUM column windows.
  - in/out degree norms are rsqrt'd on host and streamed ready-to-use.
  - GRU: 12288x4096 matrices sharded over gates, host-cast to bf16 and
    pre-transposed; gate slices combined with one small AllGather.
  - h1 exchanged with grouped AllGathers (9 tiles per collective) that
    overlap the remaining layer-1 compute.
  - Layer-2 output written column-major [64, T*512]; host un-permutes.
"""
import sys
sys.path.insert(0, '/opt/trn_rl_repo')
import numpy as np

N_CORES = 8
D = 64
VARIANT = {"g_idx": 1024, "g_q": 4, "scratch": 16384, "sp": True, "cc_g": 9,
           "sort_src": True, "no_gather": False, "no_cc": False}
BIN_COLS = 16              # columns per bin (staircase width)
BIN_SLOTS = 512            # slots per bin (128 per class)
BINS_PER_TILE = 32         # -> 512 columns, 16384 slots per tile
CPT = 128                  # chunks per tile (4 classes x 32 bins)
COLS_PER_TILE = BIN_COLS * BINS_PER_TILE


# ----------------------------------------------------------------- host prep
def _pack_core(deg, cls_cnt):
    """Assign npc nodes to bins: 4 nodes per residue class per bin, per-source
    -class slot load <= 128.  Returns (nbins, col_of_node)."""
    npc = len(deg)
    res = np.arange(npc) % 4
    need_nodes = max(int(np.ceil((np.bincount(res, minlength=4)).max() / 4)), 1)
    need_cap = int(np.ceil(cls_cnt.sum(0).max() / 125))
    nbins = max(need_nodes, need_cap)
    nbins = ((nbins + BINS_PER_TILE - 1) // BINS_PER_TILE) * BINS_PER_TILE
    while True:
        node_bin = np.full(npc, -1, np.int64)
        node_rpos = np.full(npc, -1, np.int64)
        ok = True
        for r in range(4):
            nodes_r = np.flatnonzero(res == r)
            order = nodes_r[np.argsort(-deg[nodes_r], kind="stable")]
            for rnd in range(0, len(order), nbins):
                seg = order[rnd:rnd + nbins]
                ids = np.arange(len(seg))
                if (rnd // nbins) % 2:
                    ids = nbins - 1 - ids
                node_bin[seg] = ids
                node_rpos[seg] = rnd // nbins
            if len(order) > 4 * nbins:
                ok = False
        if ok:
            loads = np.zeros((nbins, 4), np.int64)
            np.add.at(loads, node_bin, cls_cnt)
            cnts = np.zeros((nbins, 4), np.int64)
            np.add.at(cnts, (node_bin, res), 1)
            for _ in range(40000):
                viol = np.flatnonzero((loads > 128).any(1))
                if len(viol) == 0:
                    break
                b = viol[0]
                k = int(np.argmax(loads[b]))
                members = np.flatnonzero(node_bin == b)
                m = members[np.argmax(cls_cnt[members, k])]
                r = res[m]
                room = ((loads + cls_cnt[m] <= 128).all(1)
                        & (cnts[:, r] < 4))
                room[b] = False
                cand = np.flatnonzero(room)
                if len(cand) == 0:
                    ok = False
                    break
                tgt = cand[np.argmin(loads[cand].max(1))]
                loads[b] -= cls_cnt[m]; cnts[b, r] -= 1
                loads[tgt] += cls_cnt[m]; cnts[tgt, r] += 1
                node_bin[m] = tgt
            else:
                ok = False
        if ok and len(np.flatnonzero((loads > 128).any(1))) == 0:
            col = np.full(npc, -1, np.int64)
            for r in range(4):
                nodes_r = np.flatnonzero(res == r)
                order = nodes_r[np.argsort(node_bin[nodes_r], kind="stable")]
                b_sorted = node_bin[order]
                start = np.searchsorted(b_sorted, np.arange(nbins))
                within = np.arange(len(order)) - start[b_sorted]
                assert within.max() <= 3
                col[order] = node_bin[order] * BIN_COLS + within * 4 + r
            return nbins, col
        nbins += BINS_PER_TILE


def wrap16(v, pad_to=None):
    v = np.asarray(v, np.int64)
    if pad_to is not None and len(v) < pad_to:
        v = np.concatenate([v, np.zeros(pad_to - len(v), np.int64)])
    assert len(v) % 16 == 0
    w = v.reshape(-1, 16).T.astype(np.int16)
    return np.tile(w, (8, 1))


def preprocess(src, dst, n_nodes):
    npc = n_nodes // N_CORES
    deg_out = np.bincount(src, minlength=n_nodes).astype(np.int64)
    deg_in = np.bincount(dst, minlength=n_nodes).astype(np.int64)
    core_of = dst // npc
    scls = src % 4

    cols = np.empty(n_nodes, np.int64)
    nbins_c = []
    for c in range(N_CORES):
        sel = core_of == c
        dl = dst[sel] - c * npc
        cc = np.zeros((npc, 4), np.int64)
        np.add.at(cc, (dl, scls[sel]), 1)
        nb, col = _pack_core(deg_in[c * npc:(c + 1) * npc], cc)
        nbins_c.append(nb)
        cols[c * npc:(c + 1) * npc] = col
    T = (max(nbins_c) + BINS_PER_TILE - 1) // BINS_PER_TILE
    NBINS = T * BINS_PER_TILE
    C = T * CPT
    NCOL = NBINS * BIN_COLS

    # h1 layout: grouped by CC_G tiles -> [group, core, tile_in_group, col]
    G = VARIANT["cc_g"]
    NG = (T + G - 1) // G
    TP = NG * G
    tile_of = cols // COLS_PER_TILE
    cin = cols % COLS_PER_TILE
    g_of = tile_of // G
    t_in_g = tile_of % G
    pos = (g_of * (N_CORES * G * COLS_PER_TILE)
           + (np.arange(n_nodes) // npc) * (G * COLS_PER_TILE)
           + t_in_g * COLS_PER_TILE + cin)
    R_H = TP * N_CORES * COLS_PER_TILE
    assert R_H // 4 < 32768, f"h1s rows {R_H} too large for int16/4"

    out_norm = 1.0 / np.sqrt(np.maximum(deg_out, 1.0))
    in_norm = 1.0 / np.sqrt(np.maximum(deg_in, 1.0))

    cores = []
    for c in range(N_CORES):
        col = cols[c * npc:(c + 1) * npc]
        sel = core_of == c
        e_src = src[sel]
        e_col = col[dst[sel] - c * npc]
        e_cls = e_src % 4
        ebin = e_col // BIN_COLS
        key = ebin * 4 + e_cls
        order = np.argsort(key, kind="stable")
        e_src = e_src[order]; e_col = e_col[order]
        key = key[order]
        e_cls = e_src % 4
        ebin = e_col // BIN_COLS
        start = np.searchsorted(key, np.arange(NBINS * 4))
        counts = np.diff(np.append(start, len(key)))
        assert counts.max() <= 128
        NSLOT = T * 4 * BINS_PER_TILE * 128

        def layout(sec, tgt):
            # order edges within each (bin, class) chunk by the gather
            # target `sec` for HBM locality; relc follows the ordering
            if VARIANT["sort_src"]:
                o = np.lexsort((sec, key))
            else:
                o = np.arange(len(key))
            k_o = key[o]
            within = np.arange(len(k_o)) - start[k_o]
            eb = ebin[o]
            t_e = eb // BINS_PER_TILE
            g_e = eb % BINS_PER_TILE
            slot = ((t_e * 4 + e_cls[o]) * BINS_PER_TILE + g_e) * 128 + within
            idx = np.zeros(NSLOT, np.int64)
            idx[slot] = tgt[o]
            rel = np.full(NSLOT, 16.0, np.float32)
            rel[slot] = (e_col[o] % BIN_COLS).astype(np.float32)
            idx_d = wrap16(idx.reshape(T * 4, 4096)
                           .reshape(-1)).reshape(128, T * 4 * 256)
            rel_d = np.ascontiguousarray(
                rel.reshape(C, 128).T.astype(np.float32))
            return idx_d, rel_d

        idx1_d, relc1_d = layout(e_src, e_src // 4)
        idx2_d, relc2_d = layout(pos[e_src], pos[e_src] // 4)

        # norms, rsqrt'd on host
        inn = np.ones(NCOL, np.float32)
        inn[col] = in_norm[c * npc:(c + 1) * npc]
        innorm_row = inn.reshape(1, NCOL).copy()
        onr = np.ones(NCOL, np.float32)
        onr[col] = out_norm[c * npc:(c + 1) * npc]
        onorm_blk = (onr.reshape(T, 4, 128).transpose(2, 0, 1)
                     .reshape(128, T * 4).copy())
        cores.append(dict(idx1=idx1_d, idx2=idx2_d, relc1=relc1_d,
                          relc2=relc2_d, innorm_row=innorm_row,
                          onorm_blk=onorm_blk))

    return dict(T=T, C=C, NG=NG, cores=cores, pos=pos, cols=cols,
                out_norm=out_norm)


# ------------------------------------------------------------ device builder
def build_kernel(n_nodes, T, NG, hdim, repeat=1):
    import concourse.bass as bass
    import concourse.bacc as bacc
    import concourse.mybir as mybir
    import concourse.tile as tile
    from concourse.masks import make_identity

    npc = n_nodes // N_CORES
    C = T * CPT
    G = VARIANT["cc_g"]
    TP = NG * G
    R_H = TP * N_CORES * COLS_PER_TILE
    GROUP_ROWS = G * COLS_PER_TILE
    KCH = hdim // 128
    gpc = hdim // N_CORES
    MT = gpc // 128
    f32, bf16, i16, i32 = (mybir.dt.float32, mybir.dt.bfloat16,
                           mybir.dt.int16, mybir.dt.int32)
    AF = mybir.ActivationFunctionType

    nc = bacc.Bacc(dynamic_dma_scratch_size=VARIANT["scratch"],
                   num_swdge_queues=VARIANT["g_q"])
    xsc = nc.dram_tensor("xsc", [n_nodes, D], f32, kind="ExternalInput")
    wihT = nc.dram_tensor("wihT", [hdim, 3 * gpc], bf16, kind="ExternalInput")
    whhT = nc.dram_tensor("whhT", [hdim, 3 * gpc], bf16, kind="ExternalInput")
    xg = nc.dram_tensor("xg", [128, KCH, 2], bf16, kind="ExternalInput")
    hg = nc.dram_tensor("hg", [128, KCH, 2], bf16, kind="ExternalInput")
    hl = nc.dram_tensor("hl", [128, MT, 2], f32, kind="ExternalInput")
    brz = nc.dram_tensor("brz", [128, 2 * MT], f32, kind="ExternalInput")
    bnih = nc.dram_tensor("bnih", [128, MT], f32, kind="ExternalInput")
    bnhh = nc.dram_tensor("bnhh", [128, MT], f32, kind="ExternalInput")
    gb1 = nc.dram_tensor("gb1", [1, D], f32, kind="ExternalInput")
    gb2 = nc.dram_tensor("gb2", [1, D], f32, kind="ExternalInput")
    idx1_t = nc.dram_tensor("idx1", [128, T * 4 * 256], i16, kind="ExternalInput")
    idx2_t = nc.dram_tensor("idx2", [128, T * 4 * 256], i16, kind="ExternalInput")
    relc1_t = nc.dram_tensor("relc1", [128, C], f32, kind="ExternalInput")
    relc2_t = nc.dram_tensor("relc2", [128, C], f32, kind="ExternalInput")
    iota_t = nc.dram_tensor("iota", [128, BIN_COLS], f32, kind="ExternalInput")
    innorm_t = nc.dram_tensor("innorm", [1, T * COLS_PER_TILE], f32,
                              kind="ExternalInput")
    onorm_t = nc.dram_tensor("onorm", [128, T * 4], f32, kind="ExternalInput")
    out_t = nc.dram_tensor("out", [64, T * COLS_PER_TILE], f32,
                           kind="ExternalOutput")

    wbounce = nc.dram_tensor("wbounce", [gpc, 2], f32)
    wfull = nc.dram_tensor("wfull", [hdim, 2], f32, addr_space="Shared")
    h1s = nc.dram_tensor("h1s", [R_H, D], f32, addr_space="Shared")
    h1g = [nc.dram_tensor(f"h1g{g}", [GROUP_ROWS, D], f32) for g in range(NG)]
    RG = [list(range(N_CORES))]

    with tile.TileContext(nc) as tc, \
            tc.tile_pool(name="const", bufs=1) as const_pool:
      ident = const_pool.tile([64, 64], f32)
      make_identity(nc, ident[:])
      iota_sb = const_pool.tile([128, BIN_COLS], f32)
      nc.sync.dma_start(out=iota_sb[:], in_=iota_t[:])
      for _rep in range(repeat):
        with (
            tc.tile_pool(name="gru_w", bufs=3) as gru_pool,
            tc.tile_pool(name="gru_ps", bufs=1, space="PSUM") as gru_ps_pool,
            tc.tile_pool(name="gru_sb", bufs=1) as gru_sb_pool,
        ):
            # ---------------- GRU ----------------------------------------
            xg_sb = gru_sb_pool.tile([128, KCH, 2], bf16)
            hg_sb = gru_sb_pool.tile([128, KCH, 2], bf16)
            nc.sync.dma_start(out=xg_sb[:], in_=xg[:])
            nc.sync.dma_start(out=hg_sb[:], in_=hg[:])

            ps_rz = gru_ps_pool.tile([128, 4 * MT], f32)
            ps_in = gru_ps_pool.tile([128, 2 * MT], f32)
            ps_hn = gru_ps_pool.tile([128, 2 * MT], f32)
            for k in range(KCH):
                wih_k = gru_pool.tile([128, 3 * gpc], bf16, tag="wih")
                whh_k = gru_pool.tile([128, 3 * gpc], bf16, tag="whh")
                nc.sync.dma_start(out=wih_k[:], in_=wihT[k * 128:(k + 1) * 128, :])
                nc.sync.dma_start(out=whh_k[:], in_=whhT[k * 128:(k + 1) * 128, :])
                for m in range(2 * MT):
                    nc.tensor.matmul(out=ps_rz[:, 2 * m:2 * m + 2],
                                     lhsT=wih_k[:, 128 * m:128 * m + 128],
                                     rhs=xg_sb[:, k, :],
                                     start=(k == 0 and m == 0), stop=False)
                    nc.tensor.matmul(out=ps_rz[:, 2 * m:2 * m + 2],
                                     lhsT=whh_k[:, 128 * m:128 * m + 128],
                                     rhs=hg_sb[:, k, :], start=False,
                                     stop=(k == KCH - 1 and m == 2 * MT - 1))
                for m in range(MT):
                    mm = 2 * MT + m
                    nc.tensor.matmul(out=ps_in[:, 2 * m:2 * m + 2],
                                     lhsT=wih_k[:, 128 * mm:128 * mm + 128],
                                     rhs=xg_sb[:, k, :],
                                     start=(k == 0 and m == 0),
                                     stop=(k == KCH - 1 and m == MT - 1))
                    nc.tensor.matmul(out=ps_hn[:, 2 * m:2 * m + 2],
                                     lhsT=whh_k[:, 128 * mm:128 * mm + 128],
                                     rhs=hg_sb[:, k, :],
                                     start=(k == 0 and m == 0),
                                     stop=(k == KCH - 1 and m == MT - 1))

            brz_sb = gru_sb_pool.tile([128, 2 * MT], f32)
            bnih_sb = gru_sb_pool.tile([128, MT], f32)
            bnhh_sb = gru_sb_pool.tile([128, MT], f32)
            hl_sb = gru_sb_pool.tile([128, MT, 2], f32)
            nc.sync.dma_start(out=brz_sb[:], in_=brz[:])
            nc.sync.dma_start(out=bnih_sb[:], in_=bnih[:])
            nc.sync.dma_start(out=bnhh_sb[:], in_=bnhh[:])
            nc.sync.dma_start(out=hl_sb[:], in_=hl[:])
            hp = gru_sb_pool.tile([128, MT, 2], f32)
            for m in range(MT):
                r_m = gru_sb_pool.tile([128, 2], f32, tag="r_m")
                z_m = gru_sb_pool.tile([128, 2], f32, tag="z_m")
                hn_m = gru_sb_pool.tile([128, 2], f32, tag="hn_m")
                nn_m = gru_sb_pool.tile([128, 2], f32, tag="nn_m")
                nc.scalar.activation(r_m[:], ps_rz[:, 2 * m:2 * m + 2],
                                     AF.Sigmoid, bias=brz_sb[:, m:m + 1], scale=1.0)
                zi = MT + m
                nc.scalar.activation(z_m[:], ps_rz[:, 2 * zi:2 * zi + 2],
                                     AF.Sigmoid, bias=brz_sb[:, zi:zi + 1], scale=1.0)
                nc.vector.tensor_add(hn_m[:], ps_hn[:, 2 * m:2 * m + 2],
                                     bnhh_sb[:, m:m + 1].to_broadcast([128, 2]))
                nc.vector.tensor_mul(hn_m[:], r_m[:], hn_m[:])
                nc.vector.tensor_add(hn_m[:], hn_m[:], ps_in[:, 2 * m:2 * m + 2])
                nc.scalar.activation(nn_m[:], hn_m[:],
                                     AF.Tanh, bias=bnih_sb[:, m:m + 1], scale=1.0)
                t1 = gru_sb_pool.tile([128, 2], f32, tag="t1")
                nc.vector.tensor_sub(t1[:], hl_sb[:, m, :], nn_m[:])
                nc.vector.tensor_mul(t1[:], z_m[:], t1[:])
                nc.vector.tensor_add(hp[:, m, :], nn_m[:], t1[:])
            for m in range(MT):
                nc.sync.dma_start(out=wbounce[128 * m:128 * m + 128, :],
                                  in_=hp[:, m, :])
            nc.gpsimd.collective_compute(
                "AllGather", mybir.AluOpType.bypass, replica_groups=RG,
                ins=[wbounce.ap().opt()], outs=[wfull.ap().opt()])
            wf_sb = const_pool.tile([64, D, 2], f32)
            nc.sync.dma_start(
                out=wf_sb[:],
                in_=wfull.ap().rearrange("(a b) c -> a b c", b=D))
            w1_sb = const_pool.tile([64, D], bf16)
            w2_sb = const_pool.tile([64, D], bf16)
            nc.vector.tensor_copy(w1_sb[:], wf_sb[:, :, 0])
            nc.vector.tensor_copy(w2_sb[:], wf_sb[:, :, 1])
            b1_sb = const_pool.tile([64, 1], f32)
            b2_sb = const_pool.tile([64, 1], f32)
            nc.sync.dma_start(out=b1_sb[:], in_=gb1.ap().rearrange("a b -> b a"))
            nc.sync.dma_start(out=b2_sb[:], in_=gb2.ap().rearrange("a b -> b a"))

        # ---------------- GCN layers ------------------------------------
        with (
            tc.tile_pool(name="norms", bufs=1) as n_pool,
            tc.tile_pool(name="slots", bufs=2) as slot_pool,
            tc.tile_pool(name="meta", bufs=3) as meta_pool,
            tc.tile_pool(name="stp", bufs=3) as st_pool,
            tc.tile_pool(name="psA", bufs=2, space="PSUM") as psA,
            tc.tile_pool(name="psB", bufs=2, space="PSUM") as psB,
            tc.tile_pool(name="psC", bufs=2, space="PSUM") as psC,
            tc.tile_pool(name="epi", bufs=3) as epi_pool,
        ):
            onrm = n_pool.tile([128, T * 4], f32)
            nc.sync.dma_start(out=onrm[:], in_=onorm_t[:])
            relc_sbs = []
            for rt in (relc1_t, relc2_t):
                rsb = n_pool.tile([128, C], f32)
                nc.sync.dma_start(out=rsb[:], in_=rt[:])
                relc_sbs.append(rsb)

            for layer in (0, 1):
                idx_tab = idx1_t if layer == 0 else idx2_t
                relc_sb = relc_sbs[layer]
                w_sb = w1_sb if layer == 0 else w2_sb
                for t in range(T):
                    slots = slot_pool.tile([128, CPT, D], f32, tag="slots")
                    for kcl in range(4):
                        idx = meta_pool.tile([128, 256], i16, tag="idx")
                        nc.sync.dma_start(
                            out=idx[:],
                            in_=idx_tab[:, (t * 4 + kcl) * 256:(t * 4 + kcl + 1) * 256])
                        if layer == 0:
                            in_ap = (xsc.ap()
                                     .rearrange("(a b) d -> a (b d)", b=4)
                                     [:, kcl * D:(kcl + 1) * D])
                        else:
                            in_ap = (h1s.ap()
                                     .rearrange("(a b) d -> a (b d)", b=4)
                                     [:, kcl * D:(kcl + 1) * D])
                        GI = VARIANT["g_idx"]
                        ncall = 4096 // GI
                        chpc = GI // 128          # chunks per call
                        for h in range(ncall):
                            if VARIANT["no_gather"]:
                                break
                            nc.gpsimd.dma_gather(
                                out_ap=slots[:, kcl * 32 + h * chpc:
                                             kcl * 32 + (h + 1) * chpc, :],
                                in_ap=in_ap,
                                idxs_ap=idx[:, h * (GI // 16):(h + 1) * (GI // 16)],
                                num_idxs=GI, num_idxs_reg=GI,
                                elem_size=D, elem_step=4 * D,
                                single_packet=VARIANT["sp"],
                                queue_num=(t * 4 + kcl) % VARIANT["g_q"])
                    stair = st_pool.tile([128, CPT, BIN_COLS], f32, tag="stair")
                    nc.vector.tensor_tensor(
                        out=stair[:],
                        in0=relc_sb[:, t * CPT:(t + 1) * CPT]
                        .to_broadcast([128, CPT, BIN_COLS]),
                        in1=bass.AP(iota_sb[:].tensor, iota_sb[:].offset,
                                    [iota_sb[:].ap[0], [0, CPT], iota_sb[:].ap[1]]),
                        op=mybir.AluOpType.is_equal)
                    agg_ps = psA.tile([64, COLS_PER_TILE], f32, tag="agg")
                    for q in range(CPT):
                        g = q % BINS_PER_TILE
                        kcl = q // BINS_PER_TILE
                        nc.tensor.matmul(
                            out=agg_ps[:, g * BIN_COLS:(g + 1) * BIN_COLS],
                            lhsT=slots[:, kcl * 32 + g, :],
                            rhs=stair[:, kcl * 32 + g, :],
                            start=(q == 0), stop=(q == CPT - 1))
                    # per-tile in-norm row replicated across the 64 partitions
                    inrm_t_ = meta_pool.tile([64, COLS_PER_TILE], f32, tag="inrm")
                    nc.sync.dma_start(
                        out=inrm_t_[:],
                        in_=bass.AP(innorm_t.ap().tensor, t * COLS_PER_TILE,
                                    [[0, 64], [1, COLS_PER_TILE]]))
                    aggs = epi_pool.tile([64, COLS_PER_TILE], bf16, tag="aggs")
                    nc.vector.tensor_mul(aggs[:], agg_ps[:], inrm_t_[:])
                    h_ps = psB.tile([64, COLS_PER_TILE], f32, tag="h")
                    nc.tensor.matmul(out=h_ps[:], lhsT=w_sb[:], rhs=aggs[:],
                                     start=True, stop=True)
                    if layer == 0:
                        hb = epi_pool.tile([64, COLS_PER_TILE], f32, tag="hb")
                        nc.scalar.activation(hb[:], h_ps[:], AF.Relu,
                                             bias=b1_sb[:], scale=1.0)
                        hn = epi_pool.tile([128, 4, D], f32, tag="hn")
                        for b in range(4):
                            tp_ps = psC.tile([128, D], f32, tag="tp")
                            nc.tensor.transpose(out=tp_ps[:],
                                                in_=hb[:, 128 * b:128 * b + 128],
                                                identity=ident[:])
                            nc.vector.tensor_mul(
                                hn[:, b, :], tp_ps[:],
                                onrm[:, t * 4 + b:t * 4 + b + 1]
                                .to_broadcast([128, D]))
                        gi_, ti_ = t // G, t % G
                        nc.sync.dma_start(
                            out=h1g[gi_][ti_ * COLS_PER_TILE:
                                         (ti_ + 1) * COLS_PER_TILE, :]
                            .rearrange("(a p) d -> p a d", p=128),
                            in_=hn[:])
                        if (ti_ == G - 1 or t == T - 1) and not VARIANT["no_cc"]:
                            nc.gpsimd.collective_compute(
                                "AllGather", mybir.AluOpType.bypass,
                                replica_groups=RG,
                                ins=[h1g[gi_].ap().opt()],
                                outs=[h1s[gi_ * N_CORES * GROUP_ROWS:
                                          (gi_ + 1) * N_CORES * GROUP_ROWS, :].opt()])
                    else:
                        ob = epi_pool.tile([64, COLS_PER_TILE], f32, tag="ob")
                        nc.vector.tensor_add(
                            ob[:], h_ps[:],
                            b2_sb[:].to_broadcast([64, COLS_PER_TILE]))
                        nc.sync.dma_start(
                            out=out_t[:, t * COLS_PER_TILE:(t + 1) * COLS_PER_TILE],
                            in_=ob[:])
    nc.compile()
    return nc


# ------------------------------------------------------------------- driver
def make_in_maps(inputs, P, n_nodes, hdim):
    gpc = hdim // N_CORES
    KCH = hdim // 128
    MT = gpc // 128
    T = P["T"]

    X = np.stack([np.asarray(inputs["prev_gc1"]), np.asarray(inputs["prev_gc2"])], 1)
    Hm = np.stack([np.asarray(inputs["gc1_weight"]).reshape(-1),
                   np.asarray(inputs["gc2_weight"]).reshape(-1)], 1)
    from ml_dtypes import bfloat16
    xg_d = np.ascontiguousarray(
        X.reshape(KCH, 128, 2).transpose(1, 0, 2)).astype(bfloat16)
    hg_d = np.ascontiguousarray(
        Hm.reshape(KCH, 128, 2).transpose(1, 0, 2)).astype(bfloat16)

    W_ih = np.asarray(inputs["W_ih"]); W_hh = np.asarray(inputs["W_hh"])
    b_ih = np.asarray(inputs["b_ih"]); b_hh = np.asarray(inputs["b_hh"])
    emb = np.asarray(inputs["node_embeddings"], np.float32)
    xsc_d = np.ascontiguousarray(
        emb * P["out_norm"][:, None].astype(np.float32), dtype=np.float32)
    iota = np.tile(np.arange(BIN_COLS, dtype=np.float32), (128, 1))

    in_maps = []
    for c in range(N_CORES):
        rows = np.concatenate([np.arange(g * hdim + c * gpc, g * hdim + (c + 1) * gpc)
                               for g in range(3)])
        wihT_c = np.ascontiguousarray(W_ih[rows].T).astype(bfloat16)
        whhT_c = np.ascontiguousarray(W_hh[rows].T).astype(bfloat16)
        brz_c = np.ascontiguousarray(
            (b_ih[rows] + b_hh[rows])[:2 * gpc].reshape(2 * MT, 128).T, np.float32)
        bnih_c = np.ascontiguousarray(
            b_ih[rows][2 * gpc:].reshape(MT, 128).T, np.float32)
        bnhh_c = np.ascontiguousarray(
            b_hh[rows][2 * gpc:].reshape(MT, 128).T, np.float32)
        hl_c = np.ascontiguousarray(
            Hm[c * gpc:(c + 1) * gpc].reshape(MT, 128, 2).transpose(1, 0, 2),
            np.float32)
        core = P["cores"][c]
        in_maps.append({
            "xsc": xsc_d, "wihT": wihT_c, "whhT": whhT_c,
            "xg": xg_d, "hg": hg_d, "hl": hl_c,
            "brz": brz_c, "bnih": bnih_c, "bnhh": bnhh_c,
            "gb1": np.asarray(inputs["gc1_bias"], np.float32).reshape(1, D),
            "gb2": np.asarray(inputs["gc2_bias"], np.float32).reshape(1, D),
            "idx1": np.ascontiguousarray(core["idx1"]),
            "idx2": np.ascontiguousarray(core["idx2"]),
            "relc1": core["relc1"], "relc2": core["relc2"], "iota": iota,
            "innorm": core["innorm_row"], "onorm": core["onorm_blk"],
        })
    return in_maps


def kernel(node_embeddings, gc1_weight, gc2_weight, gc1_bias, gc2_bias,
           prev_gc1, prev_gc2, W_ih, W_hh, b_ih, b_hh, src, dst):
    from concourse.bass_utils import run_bass_kernel_spmd

    inputs = dict(node_embeddings=node_embeddings, gc1_weight=gc1_weight,
                  gc2_weight=gc2_weight, gc1_bias=gc1_bias, gc2_bias=gc2_bias,
                  prev_gc1=prev_gc1, prev_gc2=prev_gc2, W_ih=W_ih, W_hh=W_hh,
                  b_ih=b_ih, b_hh=b_hh, src=src, dst=dst)
    n_nodes = np.asarray(node_embeddings).shape[0]
    npc = n_nodes // N_CORES
    hdim = np.asarray(prev_gc1).shape[0]
    src = np.asarray(src); dst = np.asarray(dst)

    P = preprocess(src, dst, n_nodes)
    nc = build_kernel(n_nodes, P["T"], P["NG"], hdim)
    in_maps = make_in_maps(inputs, P, n_nodes, hdim)
    res = run_bass_kernel_spmd(nc, in_maps, core_ids=list(range(N_CORES)))
    outs = []
    for c in range(N_CORES):
        buf = np.asarray(res.results[c]["out"], np.float32)   # [64, T*512]
        cols = P["cols"][c * npc:(c + 1) * npc]
        outs.append(buf[:, cols].T)
    return np.concatenate(outs, 0).astype(np.float32)


# revision 5
# speedup vs baseline: 1.1072x; 1.1072x over previous
"""EvolveGCN (2-layer GCN + GRU weight evolution) on 8 Trainium2 NeuronCores.

Sharding: nodes/edges by destination core; GRU tensor-parallel over the
gate dim; GCN weights replicated.

  - Host pre-scales node embeddings by out_norm (f32); layer-1 gathers read
    them directly (no device x-scaling pass).
  - SpMM per core: f32 dma_gather (4 parallel SWDGE queues, indices
    sorted by target address for HBM locality) into 128-slot chunks,
    then "staircase" segment matmuls into PSUM column windows.
  - in/out degree norms are rsqrt'd on host and streamed ready-to-use.
  - GRU: 12288x4096 matrices sharded over gates, host-cast to bf16 and
    pre-transposed; gate slices combined with one small AllGather.
  - h1 exchanged with grouped AllGathers (9 tiles per collective) that
    overlap the remaining layer-1 compute.
  - Layer-2 output written column-major [64, T*512]; host un-permutes.
"""
import sys
sys.path.insert(0, '/opt/trn_rl_repo')
import numpy as np

N_CORES = 8
D = 64
VARIANT = {"g_idx": 1024, "g_q": 4, "scratch": 32768, "sp": True, "cc_g": 9,
           "sort_src": True, "no_gather": False, "no_cc": False}
BIN_COLS = 16              # columns per bin (staircase width)
BIN_SLOTS = 512            # slots per bin (128 per class)
BINS_PER_TILE = 32         # -> 512 columns, 16384 slots per tile
CPT = 128                  # chunks per tile (4 classes x 32 bins)
COLS_PER_TILE = BIN_COLS * BINS_PER_TILE


# ----------------------------------------------------------------- host prep
def _pack_core(deg, cls_cnt):
    """Assign npc nodes to bins: 4 nodes per residue class per bin, per-source
    -class slot load <= 128.  Returns (nbins, col_of_node)."""
    npc = len(deg)
    res = np.arange(npc) % 4
    need_nodes = max(int(np.ceil((np.bincount(res, minlength=4)).max() / 4)), 1)
    need_cap = int(np.ceil(cls_cnt.sum(0).max() / 125))
    nbins = max(need_nodes, need_cap)
    nbins = ((nbins + BINS_PER_TILE - 1) // BINS_PER_TILE) * BINS_PER_TILE
    while True:
        node_bin = np.full(npc, -1, np.int64)
        node_rpos = np.full(npc, -1, np.int64)
        ok = True
        for r in range(4):
            nodes_r = np.flatnonzero(res == r)
            order = nodes_r[np.argsort(-deg[nodes_r], kind="stable")]
            for rnd in range(0, len(order), nbins):
                seg = order[rnd:rnd + nbins]
                ids = np.arange(len(seg))
                if (rnd // nbins) % 2:
                    ids = nbins - 1 - ids
                node_bin[seg] = ids
                node_rpos[seg] = rnd // nbins
            if len(order) > 4 * nbins:
                ok = False
        if ok:
            loads = np.zeros((nbins, 4), np.int64)
            np.add.at(loads, node_bin, cls_cnt)
            cnts = np.zeros((nbins, 4), np.int64)
            np.add.at(cnts, (node_bin, res), 1)
            for _ in range(40000):
                viol = np.flatnonzero((loads > 128).any(1))
                if len(viol) == 0:
                    break
                b = viol[0]
                k = int(np.argmax(loads[b]))
                members = np.flatnonzero(node_bin == b)
                m = members[np.argmax(cls_cnt[members, k])]
                r = res[m]
                room = ((loads + cls_cnt[m] <= 128).all(1)
                        & (cnts[:, r] < 4))
                room[b] = False
                cand = np.flatnonzero(room)
                if len(cand) == 0:
                    ok = False
                    break
                tgt = cand[np.argmin(loads[cand].max(1))]
                loads[b] -= cls_cnt[m]; cnts[b, r] -= 1
                loads[tgt] += cls_cnt[m]; cnts[tgt, r] += 1
                node_bin[m] = tgt
            else:
                ok = False
        if ok and len(np.flatnonzero((loads > 128).any(1))) == 0:
            col = np.full(npc, -1, np.int64)
            for r in range(4):
                nodes_r = np.flatnonzero(res == r)
                order = nodes_r[np.argsort(node_bin[nodes_r], kind="stable")]
                b_sorted = node_bin[order]
                start = np.searchsorted(b_sorted, np.arange(nbins))
                within = np.arange(len(order)) - start[b_sorted]
                assert within.max() <= 3
                col[order] = node_bin[order] * BIN_COLS + within * 4 + r
            return nbins, col
        nbins += BINS_PER_TILE


def wrap16(v, pad_to=None):
    v = np.asarray(v, np.int64)
    if pad_to is not None and len(v) < pad_to:
        v = np.concatenate([v, np.zeros(pad_to - len(v), np.int64)])
    assert len(v) % 16 == 0
    w = v.reshape(-1, 16).T.astype(np.int16)
    return np.tile(w, (8, 1))


def preprocess(src, dst, n_nodes):
    npc = n_nodes // N_CORES
    deg_out = np.bincount(src, minlength=n_nodes).astype(np.int64)
    deg_in = np.bincount(dst, minlength=n_nodes).astype(np.int64)
    core_of = dst // npc
    scls = src % 4

    cols = np.empty(n_nodes, np.int64)
    nbins_c = []
    for c in range(N_CORES):
        sel = core_of == c
        dl = dst[sel] - c * npc
        cc = np.zeros((npc, 4), np.int64)
        np.add.at(cc, (dl, scls[sel]), 1)
        nb, col = _pack_core(deg_in[c * npc:(c + 1) * npc], cc)
        nbins_c.append(nb)
        cols[c * npc:(c + 1) * npc] = col
    T = (max(nbins_c) + BINS_PER_TILE - 1) // BINS_PER_TILE
    NBINS = T * BINS_PER_TILE
    C = T * CPT
    NCOL = NBINS * BIN_COLS

    # h1 layout: grouped tiles -> [group, core, tile_in_group, col].
    # Uneven groups: three big groups plus a single-tile last group, so the
    # final AllGather (the only one not hidden by layer-1 compute) is tiny.
    rest, k = T - 1, 3
    base = rest // k
    groups = [base + (1 if i < rest % k else 0) for i in range(k)] + [1]
    groups = [g for g in groups if g > 0]
    g_start = np.cumsum([0] + groups)            # per-group first tile
    tile_of = cols // COLS_PER_TILE
    cin = cols % COLS_PER_TILE
    g_of = np.searchsorted(g_start, tile_of, side="right") - 1
    t_in_g = tile_of - g_start[g_of]
    gsz = np.asarray(groups)[g_of]
    core_of_node = np.arange(n_nodes) // npc
    pos = ((g_start[g_of] * N_CORES + core_of_node * gsz + t_in_g)
           * COLS_PER_TILE + cin)
    R_H = T * N_CORES * COLS_PER_TILE
    assert R_H // 4 < 32768, f"h1s rows {R_H} too large for int16/4"

    out_norm = 1.0 / np.sqrt(np.maximum(deg_out, 1.0))
    in_norm = 1.0 / np.sqrt(np.maximum(deg_in, 1.0))

    cores = []
    for c in range(N_CORES):
        col = cols[c * npc:(c + 1) * npc]
        sel = core_of == c
        e_src = src[sel]
        e_col = col[dst[sel] - c * npc]
        e_cls = e_src % 4
        ebin = e_col // BIN_COLS
        key = ebin * 4 + e_cls
        order = np.argsort(key, kind="stable")
        e_src = e_src[order]; e_col = e_col[order]
        key = key[order]
        e_cls = e_src % 4
        ebin = e_col // BIN_COLS
        start = np.searchsorted(key, np.arange(NBINS * 4))
        counts = np.diff(np.append(start, len(key)))
        assert counts.max() <= 128
        NSLOT = T * 4 * BINS_PER_TILE * 128

        def layout(sec, tgt):
            # order edges within each (bin, class) chunk by the gather
            # target `sec` for HBM locality; relc follows the ordering
            if VARIANT["sort_src"]:
                o = np.lexsort((sec, key))
            else:
                o = np.arange(len(key))
            k_o = key[o]
            within = np.arange(len(k_o)) - start[k_o]
            eb = ebin[o]
            t_e = eb // BINS_PER_TILE
            g_e = eb % BINS_PER_TILE
            slot = ((t_e * 4 + e_cls[o]) * BINS_PER_TILE + g_e) * 128 + within
            idx = np.zeros(NSLOT, np.int64)
            idx[slot] = tgt[o]
            rel = np.full(NSLOT, 16.0, np.float32)
            rel[slot] = (e_col[o] % BIN_COLS).astype(np.float32)
            idx_d = wrap16(idx.reshape(T * 4, 4096)
                           .reshape(-1)).reshape(128, T * 4 * 256)
            rel_d = np.ascontiguousarray(
                rel.reshape(C, 128).T.astype(np.float32))
            return idx_d, rel_d

        idx1_d, relc1_d = layout(e_src, e_src // 4)
        idx2_d, relc2_d = layout(pos[e_src], pos[e_src] // 4)

        # norms, rsqrt'd on host
        inn = np.ones(NCOL, np.float32)
        inn[col] = in_norm[c * npc:(c + 1) * npc]
        innorm_row = inn.reshape(1, NCOL).copy()
        onr = np.ones(NCOL, np.float32)
        onr[col] = out_norm[c * npc:(c + 1) * npc]
        onorm_blk = (onr.reshape(T, 4, 128).transpose(2, 0, 1)
                     .reshape(128, T * 4).copy())
        cores.append(dict(idx1=idx1_d, idx2=idx2_d, relc1=relc1_d,
                          relc2=relc2_d, innorm_row=innorm_row,
                          onorm_blk=onorm_blk))

    return dict(T=T, C=C, NG=groups, cores=cores, pos=pos, cols=cols,
                out_norm=out_norm)


# ------------------------------------------------------------ device builder
def build_kernel(n_nodes, T, NG, hdim, repeat=1):
    import concourse.bass as bass
    import concourse.bacc as bacc
    import concourse.mybir as mybir
    import concourse.tile as tile
    from concourse.masks import make_identity

    npc = n_nodes // N_CORES
    C = T * CPT
    groups = list(NG)                     # tile counts per collective group
    assert sum(groups) == T
    g_start = [0]
    for g in groups:
        g_start.append(g_start[-1] + g)
    R_H = T * N_CORES * COLS_PER_TILE
    KCH = hdim // 128
    gpc = hdim // N_CORES
    MT = gpc // 128
    f32, bf16, i16, i32 = (mybir.dt.float32, mybir.dt.bfloat16,
                           mybir.dt.int16, mybir.dt.int32)
    AF = mybir.ActivationFunctionType

    nc = bacc.Bacc(dynamic_dma_scratch_size=VARIANT["scratch"],
                   num_swdge_queues=VARIANT["g_q"])
    xsc = nc.dram_tensor("xsc", [n_nodes, D], f32, kind="ExternalInput")
    wihT = nc.dram_tensor("wihT", [hdim, 3 * gpc], bf16, kind="ExternalInput")
    whhT = nc.dram_tensor("whhT", [hdim, 3 * gpc], bf16, kind="ExternalInput")
    xg = nc.dram_tensor("xg", [128, KCH, 2], bf16, kind="ExternalInput")
    hg = nc.dram_tensor("hg", [128, KCH, 2], bf16, kind="ExternalInput")
    hl = nc.dram_tensor("hl", [128, MT, 2], f32, kind="ExternalInput")
    brz = nc.dram_tensor("brz", [128, 2 * MT], f32, kind="ExternalInput")
    bnih = nc.dram_tensor("bnih", [128, MT], f32, kind="ExternalInput")
    bnhh = nc.dram_tensor("bnhh", [128, MT], f32, kind="ExternalInput")
    gb1 = nc.dram_tensor("gb1", [1, D], f32, kind="ExternalInput")
    gb2 = nc.dram_tensor("gb2", [1, D], f32, kind="ExternalInput")
    idx1_t = nc.dram_tensor("idx1", [128, T * 4 * 256], i16, kind="ExternalInput")
    idx2_t = nc.dram_tensor("idx2", [128, T * 4 * 256], i16, kind="ExternalInput")
    relc1_t = nc.dram_tensor("relc1", [128, C], f32, kind="ExternalInput")
    relc2_t = nc.dram_tensor("relc2", [128, C], f32, kind="ExternalInput")
    iota_t = nc.dram_tensor("iota", [128, BIN_COLS], f32, kind="ExternalInput")
    innorm_t = nc.dram_tensor("innorm", [1, T * COLS_PER_TILE], f32,
                              kind="ExternalInput")
    onorm_t = nc.dram_tensor("onorm", [128, T * 4], f32, kind="ExternalInput")
    out_t = nc.dram_tensor("out", [64, T * COLS_PER_TILE], f32,
                           kind="ExternalOutput")

    wbounce = nc.dram_tensor("wbounce", [gpc, 2], f32)
    wfull = nc.dram_tensor("wfull", [hdim, 2], f32, addr_space="Shared")
    h1s = nc.dram_tensor("h1s", [R_H, D], f32, addr_space="Shared")
    h1g = [nc.dram_tensor(f"h1g{g}", [gs * COLS_PER_TILE, D], f32)
           for g, gs in enumerate(groups)]
    RG = [list(range(N_CORES))]

    with tile.TileContext(nc) as tc, \
            tc.tile_pool(name="const", bufs=1) as const_pool:
      ident = const_pool.tile([64, 64], f32)
      make_identity(nc, ident[:])
      iota_sb = const_pool.tile([128, BIN_COLS], f32)
      nc.sync.dma_start(out=iota_sb[:], in_=iota_t[:])
      for _rep in range(repeat):
        with (
            tc.tile_pool(name="gru_w", bufs=3) as gru_pool,
            tc.tile_pool(name="gru_ps", bufs=1, space="PSUM") as gru_ps_pool,
            tc.tile_pool(name="gru_sb", bufs=1) as gru_sb_pool,
        ):
            # ---------------- GRU ----------------------------------------
            xg_sb = gru_sb_pool.tile([128, KCH, 2], bf16)
            hg_sb = gru_sb_pool.tile([128, KCH, 2], bf16)
            nc.sync.dma_start(out=xg_sb[:], in_=xg[:])
            nc.sync.dma_start(out=hg_sb[:], in_=hg[:])

            ps_rz = gru_ps_pool.tile([128, 4 * MT], f32)
            ps_in = gru_ps_pool.tile([128, 2 * MT], f32)
            ps_hn = gru_ps_pool.tile([128, 2 * MT], f32)
            for k in range(KCH):
                wih_k = gru_pool.tile([128, 3 * gpc], bf16, tag="wih")
                whh_k = gru_pool.tile([128, 3 * gpc], bf16, tag="whh")
                # Activation HWDGE queue: keeps the SP queue free so the GCN
                # index-table DMAs aren't stuck behind 25 MB of GRU weights
                nc.scalar.dma_start(out=wih_k[:], in_=wihT[k * 128:(k + 1) * 128, :])
                nc.scalar.dma_start(out=whh_k[:], in_=whhT[k * 128:(k + 1) * 128, :])
                for m in range(2 * MT):
                    nc.tensor.matmul(out=ps_rz[:, 2 * m:2 * m + 2],
                                     lhsT=wih_k[:, 128 * m:128 * m + 128],
                                     rhs=xg_sb[:, k, :],
                                     start=(k == 0 and m == 0), stop=False)
                    nc.tensor.matmul(out=ps_rz[:, 2 * m:2 * m + 2],
                                     lhsT=whh_k[:, 128 * m:128 * m + 128],
                                     rhs=hg_sb[:, k, :], start=False,
                                     stop=(k == KCH - 1 and m == 2 * MT - 1))
                for m in range(MT):
                    mm = 2 * MT + m
                    nc.tensor.matmul(out=ps_in[:, 2 * m:2 * m + 2],
                                     lhsT=wih_k[:, 128 * mm:128 * mm + 128],
                                     rhs=xg_sb[:, k, :],
                                     start=(k == 0 and m == 0),
                                     stop=(k == KCH - 1 and m == MT - 1))
                    nc.tensor.matmul(out=ps_hn[:, 2 * m:2 * m + 2],
                                     lhsT=whh_k[:, 128 * mm:128 * mm + 128],
                                     rhs=hg_sb[:, k, :],
                                     start=(k == 0 and m == 0),
                                     stop=(k == KCH - 1 and m == MT - 1))

            brz_sb = gru_sb_pool.tile([128, 2 * MT], f32)
            bnih_sb = gru_sb_pool.tile([128, MT], f32)
            bnhh_sb = gru_sb_pool.tile([128, MT], f32)
            hl_sb = gru_sb_pool.tile([128, MT, 2], f32)
            nc.sync.dma_start(out=brz_sb[:], in_=brz[:])
            nc.sync.dma_start(out=bnih_sb[:], in_=bnih[:])
            nc.sync.dma_start(out=bnhh_sb[:], in_=bnhh[:])
            nc.sync.dma_start(out=hl_sb[:], in_=hl[:])
            hp = gru_sb_pool.tile([128, MT, 2], f32)
            for m in range(MT):
                r_m = gru_sb_pool.tile([128, 2], f32, tag="r_m")
                z_m = gru_sb_pool.tile([128, 2], f32, tag="z_m")
                hn_m = gru_sb_pool.tile([128, 2], f32, tag="hn_m")
                nn_m = gru_sb_pool.tile([128, 2], f32, tag="nn_m")
                nc.scalar.activation(r_m[:], ps_rz[:, 2 * m:2 * m + 2],
                                     AF.Sigmoid, bias=brz_sb[:, m:m + 1], scale=1.0)
                zi = MT + m
                nc.scalar.activation(z_m[:], ps_rz[:, 2 * zi:2 * zi + 2],
                                     AF.Sigmoid, bias=brz_sb[:, zi:zi + 1], scale=1.0)
                nc.vector.tensor_add(hn_m[:], ps_hn[:, 2 * m:2 * m + 2],
                                     bnhh_sb[:, m:m + 1].to_broadcast([128, 2]))
                nc.vector.tensor_mul(hn_m[:], r_m[:], hn_m[:])
                nc.vector.tensor_add(hn_m[:], hn_m[:], ps_in[:, 2 * m:2 * m + 2])
                nc.scalar.activation(nn_m[:], hn_m[:],
                                     AF.Tanh, bias=bnih_sb[:, m:m + 1], scale=1.0)
                t1 = gru_sb_pool.tile([128, 2], f32, tag="t1")
                nc.vector.tensor_sub(t1[:], hl_sb[:, m, :], nn_m[:])
                nc.vector.tensor_mul(t1[:], z_m[:], t1[:])
                nc.vector.tensor_add(hp[:, m, :], nn_m[:], t1[:])
            for m in range(MT):
                nc.sync.dma_start(out=wbounce[128 * m:128 * m + 128, :],
                                  in_=hp[:, m, :])
            nc.gpsimd.collective_compute(
                "AllGather", mybir.AluOpType.bypass, replica_groups=RG,
                ins=[wbounce.ap().opt()], outs=[wfull.ap().opt()])
            wf_sb = const_pool.tile([64, D, 2], f32)
            nc.sync.dma_start(
                out=wf_sb[:],
                in_=wfull.ap().rearrange("(a b) c -> a b c", b=D))
            w1_sb = const_pool.tile([64, D], bf16)
            w2_sb = const_pool.tile([64, D], bf16)
            nc.vector.tensor_copy(w1_sb[:], wf_sb[:, :, 0])
            nc.vector.tensor_copy(w2_sb[:], wf_sb[:, :, 1])
            b1_sb = const_pool.tile([64, 1], f32)
            b2_sb = const_pool.tile([64, 1], f32)
            nc.sync.dma_start(out=b1_sb[:], in_=gb1.ap().rearrange("a b -> b a"))
            nc.sync.dma_start(out=b2_sb[:], in_=gb2.ap().rearrange("a b -> b a"))

        # ---------------- GCN layers ------------------------------------
        with (
            tc.tile_pool(name="norms", bufs=1) as n_pool,
            tc.tile_pool(name="slots", bufs=2) as slot_pool,
            tc.tile_pool(name="meta", bufs=3) as meta_pool,
            tc.tile_pool(name="stp", bufs=3) as st_pool,
            tc.tile_pool(name="psA", bufs=2, space="PSUM") as psA,
            tc.tile_pool(name="psB", bufs=2, space="PSUM") as psB,
            tc.tile_pool(name="psC", bufs=2, space="PSUM") as psC,
            tc.tile_pool(name="epi", bufs=3) as epi_pool,
        ):
            onrm = n_pool.tile([128, T * 4], f32)
            nc.sync.dma_start(out=onrm[:], in_=onorm_t[:])
            relc_sbs = []
            for rt in (relc1_t, relc2_t):
                rsb = n_pool.tile([128, C], f32)
                nc.sync.dma_start(out=rsb[:], in_=rt[:])
                relc_sbs.append(rsb)

            for layer in (0, 1):
                idx_tab = idx1_t if layer == 0 else idx2_t
                relc_sb = relc_sbs[layer]
                w_sb = w1_sb if layer == 0 else w2_sb
                for t in range(T):
                    slots = slot_pool.tile([128, CPT, D], f32, tag="slots")
                    for kcl in range(4):
                        idx = meta_pool.tile([128, 256], i16, tag="idx")
                        nc.sync.dma_start(
                            out=idx[:],
                            in_=idx_tab[:, (t * 4 + kcl) * 256:(t * 4 + kcl + 1) * 256])
                        if layer == 0:
                            in_ap = (xsc.ap()
                                     .rearrange("(a b) d -> a (b d)", b=4)
                                     [:, kcl * D:(kcl + 1) * D])
                        else:
                            in_ap = (h1s.ap()
                                     .rearrange("(a b) d -> a (b d)", b=4)
                                     [:, kcl * D:(kcl + 1) * D])
                        GI = VARIANT["g_idx"]
                        ncall = 4096 // GI
                        chpc = GI // 128          # chunks per call
                        for h in range(ncall):
                            if VARIANT["no_gather"]:
                                break
                            nc.gpsimd.dma_gather(
                                out_ap=slots[:, kcl * 32 + h * chpc:
                                             kcl * 32 + (h + 1) * chpc, :],
                                in_ap=in_ap,
                                idxs_ap=idx[:, h * (GI // 16):(h + 1) * (GI // 16)],
                                num_idxs=GI, num_idxs_reg=GI,
                                elem_size=D, elem_step=4 * D,
                                single_packet=VARIANT["sp"],
                                queue_num=(t * 4 + kcl) % VARIANT["g_q"])
                    stair = st_pool.tile([128, CPT, BIN_COLS], f32, tag="stair")
                    nc.vector.tensor_tensor(
                        out=stair[:],
                        in0=relc_sb[:, t * CPT:(t + 1) * CPT]
                        .to_broadcast([128, CPT, BIN_COLS]),
                        in1=bass.AP(iota_sb[:].tensor, iota_sb[:].offset,
                                    [iota_sb[:].ap[0], [0, CPT], iota_sb[:].ap[1]]),
                        op=mybir.AluOpType.is_equal)
                    agg_ps = psA.tile([64, COLS_PER_TILE], f32, tag="agg")
                    for q in range(CPT):
                        g = q % BINS_PER_TILE
                        kcl = q // BINS_PER_TILE
                        nc.tensor.matmul(
                            out=agg_ps[:, g * BIN_COLS:(g + 1) * BIN_COLS],
                            lhsT=slots[:, kcl * 32 + g, :],
                            rhs=stair[:, kcl * 32 + g, :],
                            start=(q == 0), stop=(q == CPT - 1))
                    # per-tile in-norm row replicated across the 64 partitions
                    inrm_t_ = meta_pool.tile([64, COLS_PER_TILE], f32, tag="inrm")
                    nc.sync.dma_start(
                        out=inrm_t_[:],
                        in_=bass.AP(innorm_t.ap().tensor, t * COLS_PER_TILE,
                                    [[0, 64], [1, COLS_PER_TILE]]))
                    aggs = epi_pool.tile([64, COLS_PER_TILE], bf16, tag="aggs")
                    nc.vector.tensor_mul(aggs[:], agg_ps[:], inrm_t_[:])
                    h_ps = psB.tile([64, COLS_PER_TILE], f32, tag="h")
                    nc.tensor.matmul(out=h_ps[:], lhsT=w_sb[:], rhs=aggs[:],
                                     start=True, stop=True)
                    if layer == 0:
                        hb = epi_pool.tile([64, COLS_PER_TILE], f32, tag="hb")
                        nc.scalar.activation(hb[:], h_ps[:], AF.Relu,
                                             bias=b1_sb[:], scale=1.0)
                        hn = epi_pool.tile([128, 4, D], f32, tag="hn")
                        for b in range(4):
                            tp_ps = psC.tile([128, D], f32, tag="tp")
                            nc.tensor.transpose(out=tp_ps[:],
                                                in_=hb[:, 128 * b:128 * b + 128],
                                                identity=ident[:])
                            nc.vector.tensor_mul(
                                hn[:, b, :], tp_ps[:],
                                onrm[:, t * 4 + b:t * 4 + b + 1]
                                .to_broadcast([128, D]))
                        gi_ = next(i for i in range(len(groups))
                                   if g_start[i] <= t < g_start[i + 1])
                        ti_ = t - g_start[gi_]
                        nc.sync.dma_start(
                            out=h1g[gi_][ti_ * COLS_PER_TILE:
                                         (ti_ + 1) * COLS_PER_TILE, :]
                            .rearrange("(a p) d -> p a d", p=128),
                            in_=hn[:])
                        if ti_ == groups[gi_] - 1 and not VARIANT["no_cc"]:
                            row0 = g_start[gi_] * N_CORES * COLS_PER_TILE
                            row1 = g_start[gi_ + 1] * N_CORES * COLS_PER_TILE
                            nc.gpsimd.collective_compute(
                                "AllGather", mybir.AluOpType.bypass,
                                replica_groups=RG,
                                ins=[h1g[gi_].ap().opt()],
                                outs=[h1s[row0:row1, :].opt()])
                    else:
                        ob = epi_pool.tile([64, COLS_PER_TILE], f32, tag="ob")
                        nc.vector.tensor_add(
                            ob[:], h_ps[:],
                            b2_sb[:].to_broadcast([64, COLS_PER_TILE]))
                        nc.sync.dma_start(
                            out=out_t[:, t * COLS_PER_TILE:(t + 1) * COLS_PER_TILE],
                            in_=ob[:])
    nc.compile()
    return nc


# ------------------------------------------------------------------- driver
def make_in_maps(inputs, P, n_nodes, hdim):
    gpc = hdim // N_CORES
    KCH = hdim // 128
    MT = gpc // 128
    T = P["T"]

    X = np.stack([np.asarray(inputs["prev_gc1"]), np.asarray(inputs["prev_gc2"])], 1)
    Hm = np.stack([np.asarray(inputs["gc1_weight"]).reshape(-1),
                   np.asarray(inputs["gc2_weight"]).reshape(-1)], 1)
    from ml_dtypes import bfloat16
    xg_d = np.ascontiguousarray(
        X.reshape(KCH, 128, 2).transpose(1, 0, 2)).astype(bfloat16)
    hg_d = np.ascontiguousarray(
        Hm.reshape(KCH, 128, 2).transpose(1, 0, 2)).astype(bfloat16)

    W_ih = np.asarray(inputs["W_ih"]); W_hh = np.asarray(inputs["W_hh"])
    b_ih = np.asarray(inputs["b_ih"]); b_hh = np.asarray(inputs["b_hh"])
    emb = np.asarray(inputs["node_embeddings"], np.float32)
    xsc_d = np.ascontiguousarray(
        emb * P["out_norm"][:, None].astype(np.float32), dtype=np.float32)
    iota = np.tile(np.arange(BIN_COLS, dtype=np.float32), (128, 1))

    in_maps = []
    for c in range(N_CORES):
        rows = np.concatenate([np.arange(g * hdim + c * gpc, g * hdim + (c + 1) * gpc)
                               for g in range(3)])
        wihT_c = np.ascontiguousarray(W_ih[rows].T).astype(bfloat16)
        whhT_c = np.ascontiguousarray(W_hh[rows].T).astype(bfloat16)
        brz_c = np.ascontiguousarray(
            (b_ih[rows] + b_hh[rows])[:2 * gpc].reshape(2 * MT, 128).T, np.float32)
        bnih_c = np.ascontiguousarray(
            b_ih[rows][2 * gpc:].reshape(MT, 128).T, np.float32)
        bnhh_c = np.ascontiguousarray(
            b_hh[rows][2 * gpc:].reshape(MT, 128).T, np.float32)
        hl_c = np.ascontiguousarray(
            Hm[c * gpc:(c + 1) * gpc].reshape(MT, 128, 2).transpose(1, 0, 2),
            np.float32)
        core = P["cores"][c]
        in_maps.append({
            "xsc": xsc_d, "wihT": wihT_c, "whhT": whhT_c,
            "xg": xg_d, "hg": hg_d, "hl": hl_c,
            "brz": brz_c, "bnih": bnih_c, "bnhh": bnhh_c,
            "gb1": np.asarray(inputs["gc1_bias"], np.float32).reshape(1, D),
            "gb2": np.asarray(inputs["gc2_bias"], np.float32).reshape(1, D),
            "idx1": np.ascontiguousarray(core["idx1"]),
            "idx2": np.ascontiguousarray(core["idx2"]),
            "relc1": core["relc1"], "relc2": core["relc2"], "iota": iota,
            "innorm": core["innorm_row"], "onorm": core["onorm_blk"],
        })
    return in_maps


def kernel(node_embeddings, gc1_weight, gc2_weight, gc1_bias, gc2_bias,
           prev_gc1, prev_gc2, W_ih, W_hh, b_ih, b_hh, src, dst):
    from concourse.bass_utils import run_bass_kernel_spmd

    inputs = dict(node_embeddings=node_embeddings, gc1_weight=gc1_weight,
                  gc2_weight=gc2_weight, gc1_bias=gc1_bias, gc2_bias=gc2_bias,
                  prev_gc1=prev_gc1, prev_gc2=prev_gc2, W_ih=W_ih, W_hh=W_hh,
                  b_ih=b_ih, b_hh=b_hh, src=src, dst=dst)
    n_nodes = np.asarray(node_embeddings).shape[0]
    npc = n_nodes // N_CORES
    hdim = np.asarray(prev_gc1).shape[0]
    src = np.asarray(src); dst = np.asarray(dst)

    P = preprocess(src, dst, n_nodes)
    nc = build_kernel(n_nodes, P["T"], P["NG"], hdim)
    in_maps = make_in_maps(inputs, P, n_nodes, hdim)
    res = run_bass_kernel_spmd(nc, in_maps, core_ids=list(range(N_CORES)))
    outs = []
    for c in range(N_CORES):
        buf = np.asarray(res.results[c]["out"], np.float32)   # [64, T*512]
        cols = P["cols"][c * npc:(c + 1) * npc]
        outs.append(buf[:, cols].T)
    return np.concatenate(outs, 0).astype(np.float32)


# revision 6
# speedup vs baseline: 13.4128x; 12.1137x over previous
"""EvolveGCN (2-layer GCN + GRU weight evolution) on 8 Trainium2 NeuronCores.

Sharding: nodes/edges by destination core; GRU tensor-parallel over the
gate dim; GCN weights replicated.

  - Host pre-scales node embeddings by out_norm (f32); layer-1 gathers read
    them directly (no device x-scaling pass).
  - SpMM per core: f32 dma_gather (4 parallel SWDGE queues, indices
    sorted by target address for HBM locality) into 128-slot chunks,
    then "staircase" segment matmuls into PSUM column windows.
  - in/out degree norms are rsqrt'd on host and streamed ready-to-use.
  - GRU: 12288x4096 matrices sharded over gates, host-cast to bf16 and
    pre-transposed; gate slices combined with one small AllGather.
  - h1 exchanged with grouped AllGathers (9 tiles per collective) that
    overlap the remaining layer-1 compute.
  - Layer-2 output written column-major [64, T*512]; host un-permutes.
"""
import sys
sys.path.insert(0, '/opt/trn_rl_repo')
import numpy as np

N_CORES = 8
D = 64
VARIANT = {"g_idx": 1024, "g_q": 4, "scratch": 32768, "sp": True, "cc_g": 9,
           "sort_src": True, "no_gather": False, "no_cc": False}
BIN_COLS = 16              # columns per bin (staircase width)
BIN_SLOTS = 512            # slots per bin (128 per class)
BINS_PER_TILE = 32         # -> 512 columns, 16384 slots per tile
CPT = 128                  # chunks per tile (4 classes x 32 bins)
COLS_PER_TILE = BIN_COLS * BINS_PER_TILE


# ----------------------------------------------------------------- host prep
def _pack_core(deg, cls_cnt):
    """Assign npc nodes to bins: 4 nodes per residue class per bin, per-source
    -class slot load <= 128.  Returns (nbins, col_of_node)."""
    npc = len(deg)
    res = np.arange(npc) % 4
    need_nodes = max(int(np.ceil((np.bincount(res, minlength=4)).max() / 4)), 1)
    need_cap = int(np.ceil(cls_cnt.sum(0).max() / 125))
    nbins = max(need_nodes, need_cap)
    nbins = ((nbins + BINS_PER_TILE - 1) // BINS_PER_TILE) * BINS_PER_TILE
    while True:
        node_bin = np.full(npc, -1, np.int64)
        node_rpos = np.full(npc, -1, np.int64)
        ok = True
        for r in range(4):
            nodes_r = np.flatnonzero(res == r)
            order = nodes_r[np.argsort(-deg[nodes_r], kind="stable")]
            for rnd in range(0, len(order), nbins):
                seg = order[rnd:rnd + nbins]
                ids = np.arange(len(seg))
                if (rnd // nbins) % 2:
                    ids = nbins - 1 - ids
                node_bin[seg] = ids
                node_rpos[seg] = rnd // nbins
            if len(order) > 4 * nbins:
                ok = False
        if ok:
            loads = np.zeros((nbins, 4), np.int64)
            np.add.at(loads, node_bin, cls_cnt)
            cnts = np.zeros((nbins, 4), np.int64)
            np.add.at(cnts, (node_bin, res), 1)
            for _ in range(40000):
                viol = np.flatnonzero((loads > 128).any(1))
                if len(viol) == 0:
                    break
                b = viol[0]
                k = int(np.argmax(loads[b]))
                members = np.flatnonzero(node_bin == b)
                m = members[np.argmax(cls_cnt[members, k])]
                r = res[m]
                room = ((loads + cls_cnt[m] <= 128).all(1)
                        & (cnts[:, r] < 4))
                room[b] = False
                cand = np.flatnonzero(room)
                if len(cand) == 0:
                    ok = False
                    break
                tgt = cand[np.argmin(loads[cand].max(1))]
                loads[b] -= cls_cnt[m]; cnts[b, r] -= 1
                loads[tgt] += cls_cnt[m]; cnts[tgt, r] += 1
                node_bin[m] = tgt
            else:
                ok = False
        if ok and len(np.flatnonzero((loads > 128).any(1))) == 0:
            col = np.full(npc, -1, np.int64)
            for r in range(4):
                nodes_r = np.flatnonzero(res == r)
                order = nodes_r[np.argsort(node_bin[nodes_r], kind="stable")]
                b_sorted = node_bin[order]
                start = np.searchsorted(b_sorted, np.arange(nbins))
                within = np.arange(len(order)) - start[b_sorted]
                assert within.max() <= 3
                col[order] = node_bin[order] * BIN_COLS + within * 4 + r
            return nbins, col
        nbins += BINS_PER_TILE


def wrap16(v, pad_to=None):
    v = np.asarray(v, np.int64)
    if pad_to is not None and len(v) < pad_to:
        v = np.concatenate([v, np.zeros(pad_to - len(v), np.int64)])
    assert len(v) % 16 == 0
    w = v.reshape(-1, 16).T.astype(np.int16)
    return np.tile(w, (8, 1))


def preprocess(src, dst, n_nodes):
    npc = n_nodes // N_CORES
    deg_out = np.bincount(src, minlength=n_nodes).astype(np.int64)
    deg_in = np.bincount(dst, minlength=n_nodes).astype(np.int64)
    core_of = dst // npc
    scls = src % 4

    cols = np.empty(n_nodes, np.int64)
    nbins_c = []
    for c in range(N_CORES):
        sel = core_of == c
        dl = dst[sel] - c * npc
        cc = np.zeros((npc, 4), np.int64)
        np.add.at(cc, (dl, scls[sel]), 1)
        nb, col = _pack_core(deg_in[c * npc:(c + 1) * npc], cc)
        nbins_c.append(nb)
        cols[c * npc:(c + 1) * npc] = col
    T = (max(nbins_c) + BINS_PER_TILE - 1) // BINS_PER_TILE
    NBINS = T * BINS_PER_TILE
    C = T * CPT
    NCOL = NBINS * BIN_COLS

    # h1 layout: grouped tiles -> [group, core, tile_in_group, col].
    # Uneven groups: three big groups plus a single-tile last group, so the
    # final AllGather (the only one not hidden by layer-1 compute) is tiny.
    rest, k = T - 1, 3
    base = rest // k
    groups = [base + (1 if i < rest % k else 0) for i in range(k)] + [1]
    groups = [g for g in groups if g > 0]
    g_start = np.cumsum([0] + groups)            # per-group first tile
    tile_of = cols // COLS_PER_TILE
    cin = cols % COLS_PER_TILE
    g_of = np.searchsorted(g_start, tile_of, side="right") - 1
    t_in_g = tile_of - g_start[g_of]
    gsz = np.asarray(groups)[g_of]
    core_of_node = np.arange(n_nodes) // npc
    pos = ((g_start[g_of] * N_CORES + core_of_node * gsz + t_in_g)
           * COLS_PER_TILE + cin)
    R_H = T * N_CORES * COLS_PER_TILE
    assert R_H // 4 < 32768, f"h1s rows {R_H} too large for int16/4"

    out_norm = 1.0 / np.sqrt(np.maximum(deg_out, 1.0))
    in_norm = 1.0 / np.sqrt(np.maximum(deg_in, 1.0))

    cores = []
    for c in range(N_CORES):
        col = cols[c * npc:(c + 1) * npc]
        sel = core_of == c
        e_src = src[sel]
        e_col = col[dst[sel] - c * npc]
        e_cls = e_src % 4
        ebin = e_col // BIN_COLS
        key = ebin * 4 + e_cls
        order = np.argsort(key, kind="stable")
        e_src = e_src[order]; e_col = e_col[order]
        key = key[order]
        e_cls = e_src % 4
        ebin = e_col // BIN_COLS
        start = np.searchsorted(key, np.arange(NBINS * 4))
        counts = np.diff(np.append(start, len(key)))
        assert counts.max() <= 128
        NSLOT = T * 4 * BINS_PER_TILE * 128

        def layout(sec, tgt):
            # order edges within each (bin, class) chunk by the gather
            # target `sec` for HBM locality; relc follows the ordering
            if VARIANT["sort_src"]:
                o = np.lexsort((sec, key))
            else:
                o = np.arange(len(key))
            k_o = key[o]
            within = np.arange(len(k_o)) - start[k_o]
            eb = ebin[o]
            t_e = eb // BINS_PER_TILE
            g_e = eb % BINS_PER_TILE
            slot = ((t_e * 4 + e_cls[o]) * BINS_PER_TILE + g_e) * 128 + within
            idx = np.zeros(NSLOT, np.int64)
            idx[slot] = tgt[o]
            rel = np.full(NSLOT, 16.0, np.float32)
            rel[slot] = (e_col[o] % BIN_COLS).astype(np.float32)
            idx_d = wrap16(idx.reshape(T * 4, 4096)
                           .reshape(-1)).reshape(128, T * 4 * 256)
            rel_d = np.ascontiguousarray(
                rel.reshape(C, 128).T.astype(np.float32))
            return idx_d, rel_d

        idx1_d, relc1_d = layout(e_src, e_src // 4)
        idx2_d, relc2_d = layout(pos[e_src], pos[e_src] // 4)

        # norms, rsqrt'd on host
        inn = np.ones(NCOL, np.float32)
        inn[col] = in_norm[c * npc:(c + 1) * npc]
        innorm_row = inn.reshape(1, NCOL).copy()
        onr = np.ones(NCOL, np.float32)
        onr[col] = out_norm[c * npc:(c + 1) * npc]
        onorm_blk = (onr.reshape(T, 4, 128).transpose(2, 0, 1)
                     .reshape(128, T * 4).copy())
        cores.append(dict(idx1=idx1_d, idx2=idx2_d, relc1=relc1_d,
                          relc2=relc2_d, innorm_row=innorm_row,
                          onorm_blk=onorm_blk))

    return dict(T=T, C=C, NG=groups, cores=cores, pos=pos, cols=cols,
                out_norm=out_norm)


# ------------------------------------------------------------ device builder
def build_kernel(n_nodes, T, NG, hdim, repeat=1):
    import concourse.bass as bass
    import concourse.bacc as bacc
    import concourse.mybir as mybir
    import concourse.tile as tile
    from concourse.masks import make_identity

    npc = n_nodes // N_CORES
    C = T * CPT
    groups = list(NG)                     # tile counts per collective group
    assert sum(groups) == T
    g_start = [0]
    for g in groups:
        g_start.append(g_start[-1] + g)
    R_H = T * N_CORES * COLS_PER_TILE
    KCH = hdim // 128
    gpc = hdim // N_CORES
    MT = gpc // 128
    f32, bf16, i16, i32 = (mybir.dt.float32, mybir.dt.bfloat16,
                           mybir.dt.int16, mybir.dt.int32)
    AF = mybir.ActivationFunctionType

    nc = bacc.Bacc(dynamic_dma_scratch_size=VARIANT["scratch"],
                   num_swdge_queues=VARIANT["g_q"])
    xsc = nc.dram_tensor("xsc", [n_nodes, D], f32, kind="ExternalInput")
    wihT = nc.dram_tensor("wihT", [hdim, 3 * gpc], bf16, kind="ExternalInput")
    whhT = nc.dram_tensor("whhT", [hdim, 3 * gpc], bf16, kind="ExternalInput")
    xg = nc.dram_tensor("xg", [128, KCH, 2], bf16, kind="ExternalInput")
    hg = nc.dram_tensor("hg", [128, KCH, 2], bf16, kind="ExternalInput")
    hl = nc.dram_tensor("hl", [128, MT, 2], f32, kind="ExternalInput")
    brz = nc.dram_tensor("brz", [128, 2 * MT], f32, kind="ExternalInput")
    bnih = nc.dram_tensor("bnih", [128, MT], f32, kind="ExternalInput")
    bnhh = nc.dram_tensor("bnhh", [128, MT], f32, kind="ExternalInput")
    gb1 = nc.dram_tensor("gb1", [1, D], f32, kind="ExternalInput")
    gb2 = nc.dram_tensor("gb2", [1, D], f32, kind="ExternalInput")
    idx1_t = nc.dram_tensor("idx1", [128, T * 4 * 256], i16, kind="ExternalInput")
    idx2_t = nc.dram_tensor("idx2", [128, T * 4 * 256], i16, kind="ExternalInput")
    relc1_t = nc.dram_tensor("relc1", [128, C], f32, kind="ExternalInput")
    relc2_t = nc.dram_tensor("relc2", [128, C], f32, kind="ExternalInput")
    iota_t = nc.dram_tensor("iota", [128, BIN_COLS], f32, kind="ExternalInput")
    innorm_t = nc.dram_tensor("innorm", [1, T * COLS_PER_TILE], f32,
                              kind="ExternalInput")
    onorm_t = nc.dram_tensor("onorm", [128, T * 4], f32, kind="ExternalInput")
    out_t = nc.dram_tensor("out", [64, T * COLS_PER_TILE], f32,
                           kind="ExternalOutput")

    wbounce = nc.dram_tensor("wbounce", [gpc, 2], f32)
    wfull = nc.dram_tensor("wfull", [hdim, 2], f32, addr_space="Shared")
    h1s = nc.dram_tensor("h1s", [R_H, D], f32, addr_space="Shared")
    h1g = [nc.dram_tensor(f"h1g{g}", [gs * COLS_PER_TILE, D], f32)
           for g, gs in enumerate(groups)]
    RG = [list(range(N_CORES))]

    with tile.TileContext(nc) as tc, \
            tc.tile_pool(name="const", bufs=1) as const_pool:
      ident = const_pool.tile([64, 64], f32)
      make_identity(nc, ident[:])
      iota_sb = const_pool.tile([128, BIN_COLS], f32)
      nc.sync.dma_start(out=iota_sb[:], in_=iota_t[:])
      for _rep in range(repeat):
        with (
            tc.tile_pool(name="gru_w", bufs=3) as gru_pool,
            tc.tile_pool(name="gru_ps", bufs=1, space="PSUM") as gru_ps_pool,
            tc.tile_pool(name="gru_sb", bufs=1) as gru_sb_pool,
        ):
            # ---------------- GRU ----------------------------------------
            xg_sb = gru_sb_pool.tile([128, KCH, 2], bf16)
            hg_sb = gru_sb_pool.tile([128, KCH, 2], bf16)
            nc.sync.dma_start(out=xg_sb[:], in_=xg[:])
            nc.sync.dma_start(out=hg_sb[:], in_=hg[:])

            ps_rz = gru_ps_pool.tile([128, 4 * MT], f32)
            ps_in = gru_ps_pool.tile([128, 2 * MT], f32)
            ps_hn = gru_ps_pool.tile([128, 2 * MT], f32)
            for k in range(KCH):
                wih_k = gru_pool.tile([128, 3 * gpc], bf16, tag="wih")
                whh_k = gru_pool.tile([128, 3 * gpc], bf16, tag="whh")
                # Activation HWDGE queue: keeps the SP queue free so the GCN
                # index-table DMAs aren't stuck behind 25 MB of GRU weights
                nc.scalar.dma_start(out=wih_k[:], in_=wihT[k * 128:(k + 1) * 128, :])
                nc.scalar.dma_start(out=whh_k[:], in_=whhT[k * 128:(k + 1) * 128, :])
                for m in range(2 * MT):
                    nc.tensor.matmul(out=ps_rz[:, 2 * m:2 * m + 2],
                                     lhsT=wih_k[:, 128 * m:128 * m + 128],
                                     rhs=xg_sb[:, k, :],
                                     start=(k == 0 and m == 0), stop=False)
                    nc.tensor.matmul(out=ps_rz[:, 2 * m:2 * m + 2],
                                     lhsT=whh_k[:, 128 * m:128 * m + 128],
                                     rhs=hg_sb[:, k, :], start=False,
                                     stop=(k == KCH - 1 and m == 2 * MT - 1))
                for m in range(MT):
                    mm = 2 * MT + m
                    nc.tensor.matmul(out=ps_in[:, 2 * m:2 * m + 2],
                                     lhsT=wih_k[:, 128 * mm:128 * mm + 128],
                                     rhs=xg_sb[:, k, :],
                                     start=(k == 0 and m == 0),
                                     stop=(k == KCH - 1 and m == MT - 1))
                    nc.tensor.matmul(out=ps_hn[:, 2 * m:2 * m + 2],
                                     lhsT=whh_k[:, 128 * mm:128 * mm + 128],
                                     rhs=hg_sb[:, k, :],
                                     start=(k == 0 and m == 0),
                                     stop=(k == KCH - 1 and m == MT - 1))

            brz_sb = gru_sb_pool.tile([128, 2 * MT], f32)
            bnih_sb = gru_sb_pool.tile([128, MT], f32)
            bnhh_sb = gru_sb_pool.tile([128, MT], f32)
            hl_sb = gru_sb_pool.tile([128, MT, 2], f32)
            nc.sync.dma_start(out=brz_sb[:], in_=brz[:])
            nc.sync.dma_start(out=bnih_sb[:], in_=bnih[:])
            nc.sync.dma_start(out=bnhh_sb[:], in_=bnhh[:])
            nc.sync.dma_start(out=hl_sb[:], in_=hl[:])
            hp = gru_sb_pool.tile([128, MT, 2], f32)
            for m in range(MT):
                r_m = gru_sb_pool.tile([128, 2], f32, tag="r_m")
                z_m = gru_sb_pool.tile([128, 2], f32, tag="z_m")
                hn_m = gru_sb_pool.tile([128, 2], f32, tag="hn_m")
                nn_m = gru_sb_pool.tile([128, 2], f32, tag="nn_m")
                nc.scalar.activation(r_m[:], ps_rz[:, 2 * m:2 * m + 2],
                                     AF.Sigmoid, bias=brz_sb[:, m:m + 1], scale=1.0)
                zi = MT + m
                nc.scalar.activation(z_m[:], ps_rz[:, 2 * zi:2 * zi + 2],
                                     AF.Sigmoid, bias=brz_sb[:, zi:zi + 1], scale=1.0)
                nc.vector.tensor_add(hn_m[:], ps_hn[:, 2 * m:2 * m + 2],
                                     bnhh_sb[:, m:m + 1].to_broadcast([128, 2]))
                nc.vector.tensor_mul(hn_m[:], r_m[:], hn_m[:])
                nc.vector.tensor_add(hn_m[:], hn_m[:], ps_in[:, 2 * m:2 * m + 2])
                nc.scalar.activation(nn_m[:], hn_m[:],
                                     AF.Tanh, bias=bnih_sb[:, m:m + 1], scale=1.0)
                t1 = gru_sb_pool.tile([128, 2], f32, tag="t1")
                nc.vector.tensor_sub(t1[:], hl_sb[:, m, :], nn_m[:])
                nc.vector.tensor_mul(t1[:], z_m[:], t1[:])
                nc.vector.tensor_add(hp[:, m, :], nn_m[:], t1[:])
            for m in range(MT):
                nc.sync.dma_start(out=wbounce[128 * m:128 * m + 128, :],
                                  in_=hp[:, m, :])
            nc.gpsimd.collective_compute(
                "AllGather", mybir.AluOpType.bypass, replica_groups=RG,
                ins=[wbounce.ap().opt()], outs=[wfull.ap().opt()])
            wf_sb = const_pool.tile([64, D, 2], f32)
            nc.sync.dma_start(
                out=wf_sb[:],
                in_=wfull.ap().rearrange("(a b) c -> a b c", b=D))
            w1_sb = const_pool.tile([64, D], bf16)
            w2_sb = const_pool.tile([64, D], bf16)
            nc.vector.tensor_copy(w1_sb[:], wf_sb[:, :, 0])
            nc.vector.tensor_copy(w2_sb[:], wf_sb[:, :, 1])
            b1_sb = const_pool.tile([64, 1], f32)
            b2_sb = const_pool.tile([64, 1], f32)
            nc.sync.dma_start(out=b1_sb[:], in_=gb1.ap().rearrange("a b -> b a"))
            nc.sync.dma_start(out=b2_sb[:], in_=gb2.ap().rearrange("a b -> b a"))

        # ---------------- GCN layers ------------------------------------
        with (
            tc.tile_pool(name="norms", bufs=1) as n_pool,
            tc.tile_pool(name="slots", bufs=3) as slot_pool,
            tc.tile_pool(name="meta", bufs=3) as meta_pool,
            tc.tile_pool(name="stp", bufs=3) as st_pool,
            tc.tile_pool(name="psA", bufs=2, space="PSUM") as psA,
            tc.tile_pool(name="psB", bufs=2, space="PSUM") as psB,
            tc.tile_pool(name="psC", bufs=2, space="PSUM") as psC,
            tc.tile_pool(name="epi", bufs=3) as epi_pool,
        ):
            onrm = n_pool.tile([128, T * 4], f32)
            nc.sync.dma_start(out=onrm[:], in_=onorm_t[:])
            relc_sbs = []
            for rt in (relc1_t, relc2_t):
                rsb = n_pool.tile([128, C], f32)
                nc.sync.dma_start(out=rsb[:], in_=rt[:])
                relc_sbs.append(rsb)

            for layer in (0, 1):
                idx_tab = idx1_t if layer == 0 else idx2_t
                relc_sb = relc_sbs[layer]
                w_sb = w1_sb if layer == 0 else w2_sb
                for t in range(T):
                    slots = slot_pool.tile([128, CPT, D], f32, tag="slots")
                    for kcl in range(4):
                        idx = meta_pool.tile([128, 256], i16, tag="idx")
                        nc.sync.dma_start(
                            out=idx[:],
                            in_=idx_tab[:, (t * 4 + kcl) * 256:(t * 4 + kcl + 1) * 256])
                        if layer == 0:
                            in_ap = (xsc.ap()
                                     .rearrange("(a b) d -> a (b d)", b=4)
                                     [:, kcl * D:(kcl + 1) * D])
                        else:
                            in_ap = (h1s.ap()
                                     .rearrange("(a b) d -> a (b d)", b=4)
                                     [:, kcl * D:(kcl + 1) * D])
                        GI = VARIANT["g_idx"]
                        ncall = 4096 // GI
                        chpc = GI // 128          # chunks per call
                        for h in range(ncall):
                            if VARIANT["no_gather"]:
                                break
                            nc.gpsimd.dma_gather(
                                out_ap=slots[:, kcl * 32 + h * chpc:
                                             kcl * 32 + (h + 1) * chpc, :],
                                in_ap=in_ap,
                                idxs_ap=idx[:, h * (GI // 16):(h + 1) * (GI // 16)],
                                num_idxs=GI, num_idxs_reg=GI,
                                elem_size=D, elem_step=4 * D,
                                single_packet=VARIANT["sp"],
                                queue_num=(t * 4 + kcl) % VARIANT["g_q"])
                    stair = st_pool.tile([128, CPT, BIN_COLS], f32, tag="stair")
                    nc.vector.tensor_tensor(
                        out=stair[:],
                        in0=relc_sb[:, t * CPT:(t + 1) * CPT]
                        .to_broadcast([128, CPT, BIN_COLS]),
                        in1=bass.AP(iota_sb[:].tensor, iota_sb[:].offset,
                                    [iota_sb[:].ap[0], [0, CPT], iota_sb[:].ap[1]]),
                        op=mybir.AluOpType.is_equal)
                    agg_ps = psA.tile([64, COLS_PER_TILE], f32, tag="agg")
                    for q in range(CPT):
                        g = q % BINS_PER_TILE
                        kcl = q // BINS_PER_TILE
                        nc.tensor.matmul(
                            out=agg_ps[:, g * BIN_COLS:(g + 1) * BIN_COLS],
                            lhsT=slots[:, kcl * 32 + g, :],
                            rhs=stair[:, kcl * 32 + g, :],
                            start=(q == 0), stop=(q == CPT - 1))
                    # per-tile in-norm row replicated across the 64 partitions
                    inrm_t_ = meta_pool.tile([64, COLS_PER_TILE], f32, tag="inrm")
                    nc.sync.dma_start(
                        out=inrm_t_[:],
                        in_=bass.AP(innorm_t.ap().tensor, t * COLS_PER_TILE,
                                    [[0, 64], [1, COLS_PER_TILE]]))
                    aggs = epi_pool.tile([64, COLS_PER_TILE], bf16, tag="aggs")
                    nc.vector.tensor_mul(aggs[:], agg_ps[:], inrm_t_[:])
                    h_ps = psB.tile([64, COLS_PER_TILE], f32, tag="h")
                    nc.tensor.matmul(out=h_ps[:], lhsT=w_sb[:], rhs=aggs[:],
                                     start=True, stop=True)
                    if layer == 0:
                        hb = epi_pool.tile([64, COLS_PER_TILE], f32, tag="hb")
                        nc.scalar.activation(hb[:], h_ps[:], AF.Relu,
                                             bias=b1_sb[:], scale=1.0)
                        hn = epi_pool.tile([128, 4, D], f32, tag="hn")
                        for b in range(4):
                            tp_ps = psC.tile([128, D], f32, tag="tp")
                            nc.tensor.transpose(out=tp_ps[:],
                                                in_=hb[:, 128 * b:128 * b + 128],
                                                identity=ident[:])
                            nc.vector.tensor_mul(
                                hn[:, b, :], tp_ps[:],
                                onrm[:, t * 4 + b:t * 4 + b + 1]
                                .to_broadcast([128, D]))
                        gi_ = next(i for i in range(len(groups))
                                   if g_start[i] <= t < g_start[i + 1])
                        ti_ = t - g_start[gi_]
                        nc.sync.dma_start(
                            out=h1g[gi_][ti_ * COLS_PER_TILE:
                                         (ti_ + 1) * COLS_PER_TILE, :]
                            .rearrange("(a p) d -> p a d", p=128),
                            in_=hn[:])
                        if ti_ == groups[gi_] - 1 and not VARIANT["no_cc"]:
                            row0 = g_start[gi_] * N_CORES * COLS_PER_TILE
                            row1 = g_start[gi_ + 1] * N_CORES * COLS_PER_TILE
                            nc.gpsimd.collective_compute(
                                "AllGather", mybir.AluOpType.bypass,
                                replica_groups=RG,
                                ins=[h1g[gi_].ap().opt()],
                                outs=[h1s[row0:row1, :].opt()])
                    else:
                        ob = epi_pool.tile([64, COLS_PER_TILE], f32, tag="ob")
                        nc.vector.tensor_add(
                            ob[:], h_ps[:],
                            b2_sb[:].to_broadcast([64, COLS_PER_TILE]))
                        nc.sync.dma_start(
                            out=out_t[:, t * COLS_PER_TILE:(t + 1) * COLS_PER_TILE],
                            in_=ob[:])
    nc.compile()
    return nc


# ------------------------------------------------------------------- driver
def make_in_maps(inputs, P, n_nodes, hdim):
    gpc = hdim // N_CORES
    KCH = hdim // 128
    MT = gpc // 128
    T = P["T"]

    X = np.stack([np.asarray(inputs["prev_gc1"]), np.asarray(inputs["prev_gc2"])], 1)
    Hm = np.stack([np.asarray(inputs["gc1_weight"]).reshape(-1),
                   np.asarray(inputs["gc2_weight"]).reshape(-1)], 1)
    from ml_dtypes import bfloat16
    xg_d = np.ascontiguousarray(
        X.reshape(KCH, 128, 2).transpose(1, 0, 2)).astype(bfloat16)
    hg_d = np.ascontiguousarray(
        Hm.reshape(KCH, 128, 2).transpose(1, 0, 2)).astype(bfloat16)

    W_ih = np.asarray(inputs["W_ih"]); W_hh = np.asarray(inputs["W_hh"])
    b_ih = np.asarray(inputs["b_ih"]); b_hh = np.asarray(inputs["b_hh"])
    emb = np.asarray(inputs["node_embeddings"], np.float32)
    xsc_d = np.ascontiguousarray(
        emb * P["out_norm"][:, None].astype(np.float32), dtype=np.float32)
    iota = np.tile(np.arange(BIN_COLS, dtype=np.float32), (128, 1))

    in_maps = []
    for c in range(N_CORES):
        rows = np.concatenate([np.arange(g * hdim + c * gpc, g * hdim + (c + 1) * gpc)
                               for g in range(3)])
        wihT_c = np.ascontiguousarray(W_ih[rows].T).astype(bfloat16)
        whhT_c = np.ascontiguousarray(W_hh[rows].T).astype(bfloat16)
        brz_c = np.ascontiguousarray(
            (b_ih[rows] + b_hh[rows])[:2 * gpc].reshape(2 * MT, 128).T, np.float32)
        bnih_c = np.ascontiguousarray(
            b_ih[rows][2 * gpc:].reshape(MT, 128).T, np.float32)
        bnhh_c = np.ascontiguousarray(
            b_hh[rows][2 * gpc:].reshape(MT, 128).T, np.float32)
        hl_c = np.ascontiguousarray(
            Hm[c * gpc:(c + 1) * gpc].reshape(MT, 128, 2).transpose(1, 0, 2),
            np.float32)
        core = P["cores"][c]
        in_maps.append({
            "xsc": xsc_d, "wihT": wihT_c, "whhT": whhT_c,
            "xg": xg_d, "hg": hg_d, "hl": hl_c,
            "brz": brz_c, "bnih": bnih_c, "bnhh": bnhh_c,
            "gb1": np.asarray(inputs["gc1_bias"], np.float32).reshape(1, D),
            "gb2": np.asarray(inputs["gc2_bias"], np.float32).reshape(1, D),
            "idx1": np.ascontiguousarray(core["idx1"]),
            "idx2": np.ascontiguousarray(core["idx2"]),
            "relc1": core["relc1"], "relc2": core["relc2"], "iota": iota,
            "innorm": core["innorm_row"], "onorm": core["onorm_blk"],
        })
    return in_maps


def kernel(node_embeddings, gc1_weight, gc2_weight, gc1_bias, gc2_bias,
           prev_gc1, prev_gc2, W_ih, W_hh, b_ih, b_hh, src, dst):
    from concourse.bass_utils import run_bass_kernel_spmd

    inputs = dict(node_embeddings=node_embeddings, gc1_weight=gc1_weight,
                  gc2_weight=gc2_weight, gc1_bias=gc1_bias, gc2_bias=gc2_bias,
                  prev_gc1=prev_gc1, prev_gc2=prev_gc2, W_ih=W_ih, W_hh=W_hh,
                  b_ih=b_ih, b_hh=b_hh, src=src, dst=dst)
    n_nodes = np.asarray(node_embeddings).shape[0]
    npc = n_nodes // N_CORES
    hdim = np.asarray(prev_gc1).shape[0]
    src = np.asarray(src); dst = np.asarray(dst)

    P = preprocess(src, dst, n_nodes)
    nc = build_kernel(n_nodes, P["T"], P["NG"], hdim)
    in_maps = make_in_maps(inputs, P, n_nodes, hdim)
    res = run_bass_kernel_spmd(nc, in_maps, core_ids=list(range(N_CORES)))
    outs = []
    for c in range(N_CORES):
        buf = np.asarray(res.results[c]["out"], np.float32)   # [64, T*512]
        cols = P["cols"][c * npc:(c + 1) * npc]
        outs.append(buf[:, cols].T)
    return np.concatenate(outs, 0).astype(np.float32)
